# revision 1
# baseline (speedup 1.0000x reference)
"""Deformable conv (torchvision v1, stride=1 pad=1 K=3) on 8 TRN2 NeuronCores.

Sharding: core i handles sample b=i//2, output-row half i%2 (48 of 96 rows).
Weights replicated; no cross-core communication.

Per-core pipeline (v8):
  A. offset conv (3x3, 256->18) via PE matmuls, PE-transposed to
     pixel-on-partition offs_sb [128, 36, 18] fp32.  xp buffers live in a
     scoped tile pool whose space is reused by the combine-stage pools.
  B. batched sampling math over [128, 36, 9] fp32: magic-round floor,
     validity-masked bilinear corner weights w4 [128, 36, 9, 4], single
     4-corner gather index qf = clamp(y0,-1,95)*96 + clamp(x0,-1,95) + 97,
     SWDGE-wrapped via strided DMAs into xw_all [128, 36, 9, 8] int16.
  C. per 128-px unit (36 total): dma_gather from DRAM xt4 (row q = 2KB
     [x_T[q] | x_T[q+1] | x_T[q+96] | x_T[q+97]], all 4 bilinear corners)
     -> gt [128, 9, 1024] bf16, split 1024+128 idxs (Q7 idx-scratch cap).
  D. combine: ACT expands w4 to c128 granularity (wexpc [128,9,4,128]
     bf16), DVE: one packed-2x multiply + y-fold + x-fold straight into
     val_sb [128, 4, 2304(k,ch,c)] bf16.
  E. transpose to matmul layout via PE transposes, 2 contraction tiles
     per PSUM bank, one batched PSUM->SBUF copy per pair (engine picked
     by the scheduler) -> vm [128 c, 18 ct, 512 px].
  F. 18 accumulating bf16 matmuls per (512-px block, o-half) -> psum,
     + bias via ACT, DMA out.

Measured (8 cores SPMD, device-resident repeat-delta): ~565 us/body,
rel err 5.5e-3 vs fp32 reference (gate 2e-2).
"""

import sys

import numpy as np

if "/opt/trn_rl_repo" not in sys.path:
    sys.path.insert(0, "/opt/trn_rl_repo")

import ml_dtypes  # noqa: E402

bf16 = ml_dtypes.bfloat16

B, C, H, W, O = 4, 256, 96, 96, 256
K, KK = 3, 9
HW = H * W
P = HW // 2                     # 4608 pixels per core
NCHUNK = P // 128               # 36 128-px units
NBLK = P // 512                 # 9 512-px blocks
ROWS = 48
CT = 2 * KK                     # 18 contraction tiles
MAGIC = 12582912.0              # 1.5 * 2^23
XTP_ROWS = 97 + HW + 97         # 9410
XT4_ROWS = HW + 97              # 9313 gatherable rows (q in [0, 9312])


def _wrap_idx(idx):
    """logical list -> SWDGE wrapped [128, n/16] (replicated 16-groups)."""
    n = idx.shape[0]
    w = np.zeros((128, n // 16), np.int16)
    j = np.arange(n)
    w[j % 16, j // 16] = idx
    for g in range(1, 8):
        w[g * 16 : (g + 1) * 16] = w[0:16]
    return w


# ---------------------------------------------------------------- host prep
def _prep_core_inputs(x, offset_w, offset_b, deform_w, deform_b, core):
    b, half = core // 2, core % 2
    h0 = half * ROWS
    xb = x[b]                                       # [C, H, W] fp32

    xpad = np.zeros((2, 128, 50, 98), dtype=np.float32)
    r_lo, r_hi = h0 - 1, h0 + ROWS + 1
    src_lo, src_hi = max(r_lo, 0), min(r_hi, H)
    d_lo = src_lo - r_lo
    xpad[:, :, d_lo : d_lo + (src_hi - src_lo), 1:97] = xb[
        :, src_lo:src_hi, :
    ].reshape(2, 128, src_hi - src_lo, W)
    # xp3[kx]: columns shifted by kx-1, zero-padded; rows 50, cols 96
    xp = np.stack(
        [xpad[:, :, :, kx : kx + 96] for kx in range(3)], axis=0
    ).astype(bf16)

    # 4-corner gather source: row q = [xT[q] | xT[q+1] | xT[q+96] | xT[q+97]]
    # (97-row lead pad: q = clamp(y0,-1,95)*96 + clamp(x0,-1,95) + 97)
    xtp = np.zeros((XTP_ROWS, 256), dtype=bf16)
    xtp[97 : 97 + HW] = xb.reshape(C, HW).T.astype(bf16)
    xt4 = np.concatenate(
        [
            xtp[0:XT4_ROWS],
            xtp[1 : XT4_ROWS + 1],
            xtp[96 : XT4_ROWS + 96],
            xtp[97 : XT4_ROWS + 97],
        ],
        axis=1,
    )                                               # [9313, 1024]

    ow = np.ascontiguousarray(
        offset_w.reshape(18, 2, 128, KK).transpose(1, 2, 3, 0).reshape(2, 128, KK * 18)
    ).astype(bf16)

    dw = np.ascontiguousarray(
        deform_w.reshape(O, 2, 128, KK).transpose(3, 1, 2, 0).reshape(CT, 128, O)
    ).astype(bf16)

    ob = np.ascontiguousarray(np.broadcast_to(offset_b.astype(np.float32), (128, 18)))
    db = np.ascontiguousarray(deform_b.reshape(2, 128, 1).astype(np.float32))

    p_local = np.arange(P)
    basey = (h0 + p_local // W).astype(np.float32).reshape(NCHUNK, 128).T
    basex = (p_local % W).astype(np.float32).reshape(NCHUNK, 128).T
    ky = (np.arange(KK) // 3 - 1).astype(np.float32)
    kx = (np.arange(KK) % 3 - 1).astype(np.float32)
    yb = np.ascontiguousarray(basey[:, :, None] + ky[None, None, :])  # [128,36,9]
    xb_all = np.ascontiguousarray(basex[:, :, None] + kx[None, None, :])

    identf = np.eye(128, dtype=np.float32)
    identb = np.eye(128, dtype=np.float32).astype(bf16)

    return {
        "xp": np.ascontiguousarray(xp.reshape(3, 2, 128, 50 * 96)),
        "xt4": np.ascontiguousarray(xt4),
        "ow": ow,
        "dw": dw,
        "ob": ob,
        "db": db,
        "yb": yb,
        "xb": xb_all,
        "identf": identf,
        "identb": identb,
    }


# ---------------------------------------------------------------- bass build
def build_nc(repeat=1, stages="ABCDEF", n_queues=1, mult_split=True):
    import concourse.bass as bass
    from concourse import bacc, tile

    mybir = bass.mybir
    dt = mybir.dt
    Alu = mybir.AluOpType
    Act = mybir.ActivationFunctionType

    nc = bacc.Bacc(num_swdge_queues=n_queues)

    xp_d = nc.declare_dram_parameter("xp", [3, 2, 128, 50 * 96], dt.bfloat16, isOutput=False)
    xt4_d = nc.declare_dram_parameter("xt4", [XT4_ROWS, 1024], dt.bfloat16, isOutput=False)
    ow_d = nc.declare_dram_parameter("ow", [2, 128, KK * 18], dt.bfloat16, isOutput=False)
    dw_d = nc.declare_dram_parameter("dw", [CT, 128, O], dt.bfloat16, isOutput=False)
    ob_d = nc.declare_dram_parameter("ob", [128, 18], dt.float32, isOutput=False)
    db_d = nc.declare_dram_parameter("db", [2, 128, 1], dt.float32, isOutput=False)
    yb_d = nc.declare_dram_parameter("yb", [128, NCHUNK, KK], dt.float32, isOutput=False)
    xb_d = nc.declare_dram_parameter("xb", [128, NCHUNK, KK], dt.float32, isOutput=False)
    identf_d = nc.declare_dram_parameter("identf", [128, 128], dt.float32, isOutput=False)
    identb_d = nc.declare_dram_parameter("identb", [128, 128], dt.bfloat16, isOutput=False)
    out_d = nc.declare_dram_parameter("out", [2, 128, P], dt.float32, isOutput=True)

    reg1024 = nc.gpsimd.to_reg(1024)
    reg128 = nc.gpsimd.to_reg(128)
    reg512 = nc.gpsimd.to_reg(512)

    with tile.TileContext(nc) as tc:
        from contextlib import ExitStack

        with ExitStack() as ctx:
            cst = ctx.enter_context(tc.tile_pool(name="cst", bufs=1))
            sb = ctx.enter_context(tc.tile_pool(name="sb", bufs=1))
            outp = ctx.enter_context(tc.tile_pool(name="outp", bufs=2))
            ps_t = ctx.enter_context(tc.tile_pool(name="ps_t", bufs=2, space="PSUM"))
            ps_mm = ctx.enter_context(tc.tile_pool(name="ps_mm", bufs=4, space="PSUM"))
            ps_oc = ctx.enter_context(tc.tile_pool(name="ps_oc", bufs=2, space="PSUM"))

            # ---- persistent constants / weights
            ow_sb = cst.tile([128, 2, KK * 18], dt.bfloat16)
            for t in range(2):
                nc.sync.dma_start(out=ow_sb[:, t], in_=ow_d[t])
            dw_sb = cst.tile([128, CT, O], dt.bfloat16)
            for ct_i in range(CT):
                nc.sync.dma_start(out=dw_sb[:, ct_i], in_=dw_d[ct_i])
            ob_sb = cst.tile([128, 18], dt.float32)
            nc.sync.dma_start(out=ob_sb[:], in_=ob_d[:])
            db_sb = cst.tile([128, 2], dt.float32)
            for t in range(2):
                nc.sync.dma_start(out=db_sb[:, t : t + 1], in_=db_d[t])
            yb_sb = cst.tile([128, NCHUNK, KK], dt.float32)
            nc.sync.dma_start(out=yb_sb[:], in_=yb_d[:])
            xb_sb = cst.tile([128, NCHUNK, KK], dt.float32)
            nc.sync.dma_start(out=xb_sb[:], in_=xb_d[:])
            identf_sb = cst.tile([128, 128], dt.float32)
            nc.sync.dma_start(out=identf_sb[:], in_=identf_d[:])
            identb_sb = cst.tile([128, 128], dt.bfloat16)
            nc.sync.dma_start(out=identb_sb[:], in_=identb_d[:])

            for _rep in range(repeat):
                # ---- stage A: offset conv -> offs_sb [128, 36, 18] fp32
                offs_sb = sb.tile([128, NCHUNK, 18], dt.float32, tag="offs")
                with tc.tile_pool(name="xpA", bufs=1) as xpp:
                    xp_sb = xpp.tile([128, 3, 2, 50 * 96], dt.bfloat16, tag="xp")
                    if "A" in stages:
                        for kx in range(3):
                            for t in range(2):
                                nc.sync.dma_start(
                                    out=xp_sb[:, kx, t], in_=xp_d[kx, t]
                                )
                    for g in range(NBLK if "A" in stages else 0):
                        ps = ps_oc.tile([18, 512], dt.float32, tag="psoc")
                        n = 0
                        for t in range(2):
                            for k in range(KK):
                                ky, kx = k // 3, k % 3
                                rhs = xp_sb[
                                    :, kx, t,
                                    g * 512 + ky * 96 : g * 512 + ky * 96 + 512,
                                ]
                                nc.tensor.matmul(
                                    ps[:],
                                    lhsT=ow_sb[:, t, k * 18 : (k + 1) * 18],
                                    rhs=rhs,
                                    start=(n == 0),
                                    stop=(n == 17),
                                )
                                n += 1
                        oc_sb = sb.tile([18, 512], dt.float32, tag="ocsb")
                        nc.scalar.copy(oc_sb[:], ps[:])
                        for t3 in range(4):
                            pst = ps_t.tile([128, 18], dt.float32, tag="pst")
                            nc.tensor.transpose(
                                pst[:],
                                oc_sb[:, t3 * 128 : (t3 + 1) * 128],
                                identf_sb[:18, :18],
                            )
                            nc.vector.tensor_tensor(
                                offs_sb[:, g * 4 + t3], pst[:], ob_sb[:], Alu.add
                            )

                # ---- stage B: sampling math over [128, 36, 9] fp32
                scr = sb.tile([128, 9, NCHUNK, KK], dt.float32, tag="scr")
                w4 = sb.tile([128, NCHUNK, KK, 4], dt.float32, tag="w4")
                qf_sb = sb.tile([128, NCHUNK, KK], dt.float32, tag="qf")
                qi_sb = sb.tile([128, NCHUNK, KK], dt.int16, tag="qi")
                xw_all = sb.tile([128, NCHUNK, KK, 8], dt.int16, tag="xw")

                for _sB in range(1 if "B" in stages else 0):
                    py, px = scr[:, 0], scr[:, 1]
                    ty, tx = scr[:, 2], scr[:, 3]
                    y0, x0 = scr[:, 4], scr[:, 5]
                    t1 = scr[:, 6]
                    va, vb = scr[:, 7], scr[:, 8]

                    dy = offs_sb.rearrange(
                        "p n (k two) -> p n k two", two=2
                    )[:, :, :, 0]
                    dx = offs_sb.rearrange(
                        "p n (k two) -> p n k two", two=2
                    )[:, :, :, 1]
                    nc.vector.tensor_tensor(py, dy, yb_sb[:], Alu.add)
                    nc.vector.tensor_tensor(px, dx, xb_sb[:], Alu.add)
                    for (pp, tt, zz) in ((py, ty, y0), (px, tx, x0)):
                        nc.vector.tensor_scalar(
                            out=t1, in0=pp, scalar1=0.49999997, scalar2=MAGIC,
                            op0=Alu.subtract, op1=Alu.add,
                        )
                        nc.vector.tensor_scalar(
                            out=zz, in0=t1, scalar1=MAGIC, scalar2=None,
                            op0=Alu.subtract,
                        )
                        nc.vector.tensor_tensor(tt, pp, zz, Alu.subtract)

                    # corner weights -> w4[p, u, k, j]; j = (ycorner, xcorner)
                    # va = vy0 = (0<=y0<=95)*(1-ty); vb = vy1 = (-1<=y0<=94)*ty
                    nc.vector.tensor_scalar(out=t1, in0=y0, scalar1=0.0, scalar2=None, op0=Alu.is_ge)
                    nc.vector.tensor_scalar(out=va, in0=y0, scalar1=95.0, scalar2=None, op0=Alu.is_le)
                    nc.vector.tensor_tensor(va, va, t1, Alu.mult)
                    nc.vector.tensor_scalar(out=t1, in0=ty, scalar1=-1.0, scalar2=1.0, op0=Alu.mult, op1=Alu.add)
                    nc.vector.tensor_tensor(va, va, t1, Alu.mult)
                    nc.vector.tensor_scalar(out=t1, in0=y0, scalar1=-1.0, scalar2=None, op0=Alu.is_ge)
                    nc.vector.tensor_scalar(out=vb, in0=y0, scalar1=94.0, scalar2=None, op0=Alu.is_le)
                    nc.vector.tensor_tensor(vb, vb, t1, Alu.mult)
                    nc.vector.tensor_tensor(vb, vb, ty, Alu.mult)
                    # t1 = wx0, ty(reused) = wx1
                    wx0, wx1 = scr[:, 6], scr[:, 2]
                    nc.vector.tensor_scalar(out=wx0, in0=x0, scalar1=0.0, scalar2=None, op0=Alu.is_ge)
                    nc.vector.tensor_scalar(out=py, in0=x0, scalar1=95.0, scalar2=None, op0=Alu.is_le)
                    nc.vector.tensor_tensor(wx0, wx0, py, Alu.mult)
                    nc.vector.tensor_scalar(out=py, in0=tx, scalar1=-1.0, scalar2=1.0, op0=Alu.mult, op1=Alu.add)
                    nc.vector.tensor_tensor(wx0, wx0, py, Alu.mult)
                    nc.vector.tensor_scalar(out=wx1, in0=x0, scalar1=-1.0, scalar2=None, op0=Alu.is_ge)
                    nc.vector.tensor_scalar(out=py, in0=x0, scalar1=94.0, scalar2=None, op0=Alu.is_le)
                    nc.vector.tensor_tensor(wx1, wx1, py, Alu.mult)
                    nc.vector.tensor_tensor(wx1, wx1, tx, Alu.mult)

                    nc.vector.tensor_tensor(w4[:, :, :, 0], va, wx0, Alu.mult)
                    nc.vector.tensor_tensor(w4[:, :, :, 1], va, wx1, Alu.mult)
                    nc.vector.tensor_tensor(w4[:, :, :, 2], vb, wx0, Alu.mult)
                    nc.vector.tensor_tensor(w4[:, :, :, 3], vb, wx1, Alu.mult)

                    # gather index: clamp(y0,-1,95)*96 + clamp(x0,-1,95) + 97
                    nc.vector.tensor_scalar(out=va, in0=y0, scalar1=-1.0, scalar2=95.0, op0=Alu.max, op1=Alu.min)
                    nc.vector.tensor_scalar(out=vb, in0=x0, scalar1=-1.0, scalar2=95.0, op0=Alu.max, op1=Alu.min)
                    nc.vector.tensor_scalar(out=vb, in0=vb, scalar1=97.0, scalar2=None, op0=Alu.add)
                    nc.vector.scalar_tensor_tensor(
                        out=qf_sb[:], in0=va, scalar=96.0, in1=vb,
                        op0=Alu.mult, op1=Alu.add,
                    )
                    nc.vector.tensor_copy(qi_sb[:], qf_sb[:])
                    # SWDGE wrap: idx j = k*128 + pp  ->  [j%16, u, j//16]
                    # col layout per unit: [k, g8 = pp//16]
                    for g8 in range(8):
                        nc.sync.dma_start(
                            out=xw_all[0:16, :, :, g8],
                            in_=qi_sb[g8 * 16 : (g8 + 1) * 16],
                        )
                    for g8 in range(1, 8):
                        nc.sync.dma_start(
                            out=xw_all[g8 * 16 : (g8 + 1) * 16], in_=xw_all[0:16]
                        )

                # ---- stages C-F (pools scoped per rep: reuse stage-A space)
                # queue = global Pool-DMA index % n_queues keeps Tile's 8
                # round-robin DMASW sem lanes each pinned to one SWDGE queue.
                pool_dma_idx = 0
                cf = ExitStack()
                g_pool = cf.enter_context(tc.tile_pool(name="gth", bufs=3))
                wex_pool = cf.enter_context(tc.tile_pool(name="wex", bufs=3))
                m_pool = cf.enter_context(tc.tile_pool(name="mt", bufs=1))
                val_pool = cf.enter_context(tc.tile_pool(name="val", bufs=1))
                vm_pool = cf.enter_context(tc.tile_pool(name="vm", bufs=2))
                gt_static = None
                val_static = None
                vm_static = None
                if "C" not in stages:
                    gt_static = g_pool.tile([128, KK, 1024], dt.bfloat16, tag="g")
                    nc.vector.memset(gt_static[:], 0)
                if "D" not in stages:
                    val_static = val_pool.tile(
                        [128, 4, CT * 128], dt.bfloat16, tag="val"
                    )
                    nc.vector.memset(val_static[:], 0)
                if "E" not in stages:
                    vm_static = vm_pool.tile([128, CT, 512], dt.bfloat16, tag="vm")
                    nc.vector.memset(vm_static[:], 0)

                for Bb in range(NBLK):
                    val_sb = (
                        val_static
                        if val_static is not None
                        else val_pool.tile([128, 4, CT * 128], dt.bfloat16, tag="val")
                    )
                    for qc in range(4):               # 128-px units
                        u = Bb * 4 + qc
                        gt = (
                            gt_static
                            if gt_static is not None
                            else g_pool.tile([128, KK, 1024], dt.bfloat16, tag="g")
                        )
                        if "C" in stages:
                            # Q7 idx scratch caps num_idxs at 1024: taps 0-7
                            # in one gather, tap 8 in a second small one.
                            nc.gpsimd.dma_gather(
                                out_ap=gt[:, 0:8],
                                in_ap=xt4_d[:],
                                idxs_ap=xw_all[:, u, 0:8].rearrange(
                                    "p k g -> p (k g)"
                                ),
                                num_idxs=1024,
                                num_idxs_reg=reg1024,
                                elem_size=1024,
                                queue_num=pool_dma_idx % n_queues,
                            )
                            pool_dma_idx += 1
                            nc.gpsimd.dma_gather(
                                out_ap=gt[:, 8:9],
                                in_ap=xt4_d[:],
                                idxs_ap=xw_all[:, u, 8],
                                num_idxs=128,
                                num_idxs_reg=reg128,
                                elem_size=1024,
                                queue_num=pool_dma_idx % n_queues,
                            )
                            pool_dma_idx += 1
                        if "D" in stages:
                            # ACT: expand corner weights to c128 granularity
                            wexpc = wex_pool.tile(
                                [128, KK, 4, 128], dt.bfloat16, tag="wex"
                            )
                            nc.scalar.copy(
                                wexpc[:],
                                w4[:, u].unsqueeze(3).broadcast_to(
                                    (128, KK, 4, 128)
                                ),
                            )
                            # DVE: packed-2x multiply + y-fold + x-fold
                            m = m_pool.tile(
                                [128, KK, 4, 2, 128], dt.bfloat16, tag="m"
                            )
                            # two stride-0-free ops (one per c-half) keep the
                            # DVE in packed-2x mode; a single broadcast op
                            # falls back to 1x.
                            if mult_split:
                                for chh in range(2):
                                    nc.vector.tensor_tensor(
                                        m[:, :, :, chh],
                                        gt.rearrange(
                                            "p k (j ch c) -> p k j ch c",
                                            j=4, ch=2,
                                        )[:, :, :, chh],
                                        wexpc[:],
                                        Alu.mult,
                                    )
                            else:
                                nc.vector.tensor_tensor(
                                    m[:],
                                    gt.rearrange(
                                        "p k (j ch c) -> p k j ch c", j=4, ch=2
                                    ),
                                    wexpc.unsqueeze(3).broadcast_to(
                                        (128, KK, 4, 2, 128)
                                    ),
                                    Alu.mult,
                                )
                            s = m_pool.tile([128, KK, 2, 2, 128], dt.bfloat16, tag="s")
                            nc.vector.tensor_tensor(
                                s[:], m[:, :, 0:2], m[:, :, 2:4], Alu.add
                            )
                            nc.vector.tensor_tensor(
                                val_sb[:, qc].rearrange(
                                    "p (k ch c) -> p k ch c", k=KK, ch=2
                                ),
                                s[:, :, 0],
                                s[:, :, 1],
                                Alu.add,
                            )
                    # transpose to matmul layout via PE transposes + ACT copies
                    vm = (
                        vm_static
                        if vm_static is not None
                        else vm_pool.tile([128, CT, 512], dt.bfloat16, tag="vm")
                    )
                    if "E" in stages:
                        # 2 ct per PSUM bank (bf16 [128,1024] = 2KB): 8 PE
                        # transposes then one batched PSUM->SBUF copy.
                        for ct2 in range(CT // 2):
                            ptr = ps_mm.tile([128, 2, 512], dt.bfloat16, tag="pm")
                            for half in range(2):
                                ct_i = ct2 * 2 + half
                                for qc in range(4):
                                    nc.tensor.transpose(
                                        ptr[:, half, qc * 128 : (qc + 1) * 128],
                                        val_sb[
                                            :, qc, ct_i * 128 : (ct_i + 1) * 128
                                        ],
                                        identb_sb[:],
                                    )
                            nc.any.tensor_copy(
                                vm[:, ct2 * 2 : ct2 * 2 + 2], ptr[:]
                            )
                    for oh in range(2 if "F" in stages else 0):
                        pm = ps_mm.tile([128, 512], dt.float32, tag="pm")
                        for ct_i in range(CT):
                            nc.tensor.matmul(
                                pm[:],
                                lhsT=dw_sb[:, ct_i, oh * 128 : (oh + 1) * 128],
                                rhs=vm[:, ct_i],
                                start=(ct_i == 0),
                                stop=(ct_i == CT - 1),
                            )
                        ob_t = outp.tile([128, 512], dt.float32, tag="ot")
                        nc.scalar.activation(
                            out=ob_t[:], in_=pm[:],
                            func=Act.Identity, bias=db_sb[:, oh : oh + 1], scale=1.0,
                        )
                        nc.sync.dma_start(
                            out=out_d[oh, :, Bb * 512 : (Bb + 1) * 512], in_=ob_t[:]
                        )
                cf.close()

    nc.compile()
    return nc


# ------------------------------------------------------------ main entry
_NC_CACHE = {}


def _get_nc():
    if "nc" not in _NC_CACHE:
        _NC_CACHE["nc"] = build_nc()
    return _NC_CACHE["nc"]


def _assemble(results):
    out = np.empty((B, O, H, W), dtype=np.float32)
    for core in range(8):
        b, half = core // 2, core % 2
        o = np.asarray(results[core]["out"]).reshape(O, ROWS, W)
        out[b, :, half * ROWS : (half + 1) * ROWS, :] = o
    return out


def kernel(x, offset_w, offset_b, deform_w, deform_b, **_ignored):
    from concourse.bass_utils import run_bass_kernel_spmd

    x = np.asarray(x, dtype=np.float32)
    offset_w = np.asarray(offset_w, dtype=np.float32)
    offset_b = np.asarray(offset_b, dtype=np.float32)
    deform_w = np.asarray(deform_w, dtype=np.float32)
    deform_b = np.asarray(deform_b, dtype=np.float32)

    nc = _get_nc()
    in_maps = [
        _prep_core_inputs(x, offset_w, offset_b, deform_w, deform_b, core)
        for core in range(8)
    ]
    res = run_bass_kernel_spmd(nc, in_maps, core_ids=list(range(8)))
    return _assemble([res.results[i] for i in range(8)])

